# revision 34
# baseline (speedup 1.0000x reference)
"""Multi-head gated axial attention (width axis) — Trainium2 Bass kernel.

Problem: nn_MultiHeadGatedAxialAttentionWidth_63582695850407
Shapes: x (4, 256, 64, 128); wq/wk/wv/wout (256, 256); rq/rk/rv
(8, 32, 128, 128); Gq/Gk/Gv1/Gv2 (8,).

Sharding: 8 cores = (batch n ∈ 0..3) × (head-group hp ∈ 0..1, 4 heads each).
Each core computes q/k/v convs for its batch + head-group, the axial
attention over the width axis (including the per-position relative-bias
terms), and the gated value mixes.  The final 1x1 conv needs all 256
channels, so the two head-group cores of each batch AllGather their mixed
outputs (bf16, 2 MB) and both compute the full output conv for the batch
(hp=0 core's output is used by the host).

All matmul operands are bf16 (fp32 PSUM accumulate); tanh of the relative
tables and all per-head gate folds are done on the host (pure input prep).
Layout notes: per-core pixel order is (j, i) [width-major] so the per-j
relative matmuls read contiguous q/k slabs.  Logits accumulate in PSUM as
L[w, j, i] (w = key position on partitions): qk uses k-row stationaries,
qrq/krk use per-j relative-table stationaries, all accumulating into the
same PSUM chunk.  Softmax is un-normalized exp (logits are O(±4), no max
subtraction needed); the 1/sum normalization is folded into the PSUM
evacuation of the x1+x2 value mix (sum replicated across partitions via a
GPSIMD partition all-reduce of exp(L)).
"""

import os
import sys

import numpy as np

sys.path.insert(0, "/opt/trn_rl_repo")
sys.path.insert(0, "/opt/pypackages")

N, C, H, W = 4, 256, 64, 128
NH = 8
HD = C // NH  # 32
D = float(np.sqrt(C))  # 16
NCORES = 8
PIX = H * W  # 8192 per batch

_CACHE = {}


def _build_nc():
    import concourse.bass as bass
    import concourse.mybir as mybir
    import concourse.tile as tile
    from contextlib import ExitStack

    f32 = mybir.dt.float32
    bf = mybir.dt.bfloat16
    AF = mybir.ActivationFunctionType
    ALU = mybir.AluOpType

    nc = bass.Bass("TRN2", target_bir_lowering=False, debug=False, num_devices=8)

    xs_d = nc.dram_tensor("xs", [2, 128, PIX], bf, kind="ExternalInput").ap()
    rq_d = nc.dram_tensor("rqs", [128, W, W], bf, kind="ExternalInput").ap()
    rk_d = nc.dram_tensor("rks", [128, W, W], bf, kind="ExternalInput").ap()
    rvt_d = nc.dram_tensor("rvt", [128, 4, W, HD], bf, kind="ExternalInput").ap()
    wc_d = nc.dram_tensor("wcat", [2, 128, 384], bf, kind="ExternalInput").ap()
    wo_d = nc.dram_tensor("wo", [2, 128, 256], bf, kind="ExternalInput").ap()
    id_d = nc.dram_tensor("ident", [128, 32], bf, kind="ExternalInput").ap()
    yo_d = nc.dram_tensor("yo", [2, 128, PIX], bf, kind="ExternalOutput").ap()

    with tile.TileContext(nc) as tc, ExitStack() as ctx:
        # DMA-landing pools opened first so they occupy fresh, never-reused
        # SBUF (a DMA that is the first toucher of a recycled pool region
        # inherits the region's release waits, and the HW DMA descriptor
        # only has room for 2).
        xbp = ctx.enter_context(tc.tile_pool(name="xbp", bufs=4))
        rqk = ctx.enter_context(tc.tile_pool(name="rqk", bufs=2))
        sxp = ctx.enter_context(tc.tile_pool(name="sxp", bufs=1))
        const = ctx.enter_context(tc.tile_pool(name="const", bufs=1))
        dramp = ctx.enter_context(tc.tile_pool(name="dram", bufs=1, space="DRAM"))

        wc = const.tile([128, 2, 384], bf)
        wo = const.tile([128, 2, 256], bf)
        ident = const.tile([128, 32], bf)
        ones = const.tile([128, 128], bf)
        for kt in range(2):
            nc.sync.dma_start(out=wc[:, kt, :], in_=wc_d[kt], single_packet=True)
            nc.sync.dma_start(out=wo[:, kt, :], in_=wo_d[kt], single_packet=True)
        nc.sync.dma_start(out=ident, in_=id_d, single_packet=True)
        nc.vector.memset(ones, 1.0)

        sx = [sxp.tile([128, 64, H], bf, tag=f"sx{t}", name=f"sx{t}")
              for t in range(2)]  # [(hl,c), j-half, i]

        mid = ExitStack()
        vtp = mid.enter_context(tc.tile_pool(name="vt", bufs=1))
        VT = vtp.tile([128, 4, H, HD], bf)  # [w, hl, i, c]
        mid2 = ExitStack()
        ep = mid2.enter_context(tc.tile_pool(name="ep", bufs=1))
        E = [ep.tile([128, W, H], bf, tag=f"E{hl}", name=f"E{hl}")
             for hl in range(4)]  # [w, j, i]
        qks = ExitStack()
        qkvp = qks.enter_context(tc.tile_pool(name="qkv", bufs=1))
        q_sb = qkvp.tile([128, W, H], bf, tag="q")  # [chan, j, i]
        k_sb = qkvp.tile([128, W, H], bf, tag="k")
        vs = ExitStack()
        vp = vs.enter_context(tc.tile_pool(name="vp", bufs=1))
        v_sb = vp.tile([128, H, W], bf, tag="v")  # [chan, i, j]
        dsts = (q_sb, k_sb, v_sb.rearrange("p i j -> p j i"))

        xsT = xs_d.rearrange("k p x -> p k x")

        # ---- phase 1: q/k/v 1x1 convs ----------------------------------
        with tc.tile_pool(name="cps", bufs=4, space="PSUM") as cps:
            for px in range(16):  # 512-pixel blocks
                xb = xbp.tile([128, 2, 512], bf, tag="xb")
                nc.sync.dma_start(
                    out=xb, in_=xsT[:, :, px * 512:(px + 1) * 512],
                    single_packet=True,
                )
                for m in range(3):
                    ps = cps.tile([128, 512], f32, tag="cv")
                    for kt in range(2):
                        nc.tensor.matmul(
                            ps,
                            lhsT=wc[:, kt, m * 128:(m + 1) * 128],
                            rhs=xb[:, kt, :],
                            start=(kt == 0),
                            stop=(kt == 1),
                        )
                    dst = dsts[m][:, px * 8:(px + 1) * 8, :]
                    if m == 0:
                        nc.scalar.copy(dst, ps)
                    else:
                        nc.vector.tensor_copy(dst, ps)

        # ---- phase 2: V^T tiles via PE transpose -----------------------
        with tc.tile_pool(name="tps", bufs=4, space="PSUM") as tps:
            for hl in range(4):
                c0 = 32 * hl
                for i in range(H):
                    tp = tps.tile([128, HD], bf, tag="tp")
                    nc.tensor.transpose(
                        tp, in_=v_sb[c0:c0 + 32, i, :],
                        identity=ident[c0:c0 + 32, :],
                        tile_position=(c0, 0),
                    )
                    dst = VT[:, hl, i, :]
                    if i % 2 == 0:
                        nc.scalar.copy(dst, tp)
                    else:
                        nc.vector.tensor_copy(dst, tp)
        vs.close()

        # ---- phase 3: logits + exp per head ----------------------------
        # Two PSUM accumulators per (jq, head, i-half) chunk: L_r (qrq+krk,
        # per-j rows) and L_q (qk, per-i rows) — their output row
        # orientations are transposed, so they can't share a PSUM tile with
        # bank-local matmul writes.  Merged via exp(a+b) = exp(a)*exp(b).
        with tc.tile_pool(name="etp", bufs=2) as etp, \
             tc.tile_pool(name="lps", bufs=2, space="PSUM") as lps:
            for jq in range(4):  # 32-j table chunks (shared by all heads)
                rqc = rqk.tile([128, 32, W], bf, tag="rq")
                rkc = rqk.tile([128, 32, W], bf, tag="rk")
                nc.sync.dma_start(out=rqc, in_=rq_d[:, jq * 32:(jq + 1) * 32, :],
                                  single_packet=True)
                nc.sync.dma_start(out=rkc, in_=rk_d[:, jq * 32:(jq + 1) * 32, :],
                                  single_packet=True)
                for hl in range(4):
                    c0 = 32 * hl
                    for ih in range(2):  # 32-i halves
                        i0 = ih * 32
                        Lr = lps.tile([128, 32, 32], f32, tag="Lr")
                        for jl in range(32):
                            j = jq * 32 + jl
                            nc.tensor.matmul(
                                Lr[:, jl, :],
                                lhsT=rqc[c0:c0 + 32, jl, :],
                                rhs=q_sb[c0:c0 + 32, j, i0:i0 + 32],
                                start=(jl % 16 == 0), stop=False,
                                tile_position=(c0, 0),
                            )
                            nc.tensor.matmul(
                                Lr[:, jl, :],
                                lhsT=rkc[c0:c0 + 32, jl, :],
                                rhs=k_sb[c0:c0 + 32, j, i0:i0 + 32],
                                start=False, stop=(jl % 16 == 15),
                                tile_position=(c0, 0),
                            )
                        Lq = lps.tile([128, 32, 32], f32, tag="Lq")
                        for il in range(32):
                            i = i0 + il
                            nc.tensor.matmul(
                                Lq[:, il, :],
                                lhsT=k_sb[c0:c0 + 32, :, i],
                                rhs=q_sb[c0:c0 + 32, jq * 32:(jq + 1) * 32, i],
                                start=(il % 16 == 0), stop=(il % 16 == 15),
                                tile_position=(c0, 0),
                            )
                        Ert = etp.tile([128, 32, 32], bf, tag="Ert")
                        Eqt = etp.tile([128, 32, 32], bf, tag="Eqt")
                        nc.scalar.activation(Ert, Lr, AF.Exp)
                        nc.scalar.activation(Eqt, Lq, AF.Exp)
                        nc.vector.tensor_tensor(
                            out=E[hl][:, jq * 32:(jq + 1) * 32, i0:i0 + 32],
                            in0=Ert,
                            in1=Eqt.rearrange("p i j -> p j i"),
                            op=ALU.mult,
                        )

        # ---- phase 3b: softmax sums via ones-matmul --------------------
        # s[j,i] = sum_w E[w,j,i], replicated across partitions by an
        # all-ones stationary; sliced per head block into sx[t].
        with tc.tile_pool(name="srp", bufs=1) as srp, \
             tc.tile_pool(name="sps", bufs=4, space="PSUM") as sps:
            for t in range(2):
                for hl in range(4):
                    c0 = 32 * hl
                    s_repa = srp.tile([128, 32 * H], bf, tag="srepa")
                    s_repb = srp.tile([128, 32 * H], bf, tag="srepb")
                    Eh = E[hl][:, t * 64:(t + 1) * 64, :].rearrange(
                        "p j i -> p (j i)")
                    for px in range(8):
                        sp = sps.tile([128, 512], f32, tag="sp")
                        nc.tensor.matmul(
                            sp, lhsT=ones,
                            rhs=Eh[:, px * 512:(px + 1) * 512],
                            start=True, stop=True,
                        )
                        if px < 4:
                            nc.scalar.copy(
                                s_repa[:, px * 512:(px + 1) * 512], sp)
                        else:
                            nc.vector.tensor_copy(
                                s_repb[:, (px - 4) * 512:(px - 3) * 512], sp)
                    sxv = sx[t][c0:c0 + 32, :, :].rearrange("p j i -> p (j i)")
                    nc.sync.dma_start(out=sxv[:, 0:2048], in_=s_repa[0:32, :],
                                      single_packet=True)
                    nc.sync.dma_start(out=sxv[:, 2048:4096], in_=s_repb[0:32, :],
                                      single_packet=True)
        qks.close()

        # ---- phase 4: x1 + x2 value mix, normalized --------------------
        # x1 (per-i rows) and x2 (per-j rows) need transposed PSUM layouts,
        # so they accumulate in separate PSUM passes, each divided by the
        # softmax sum on evacuation and merged in SBUF.
        with tc.tile_pool(name="xnp", bufs=1) as xnp, \
             tc.tile_pool(name="tup", bufs=2) as tup, \
             tc.tile_pool(name="xps", bufs=1, space="PSUM") as xps:
            Xn = xnp.tile([128, W, H], bf)  # [(hl,c), j, i]
            for t in range(2):  # j halves
                rx = tup.tile([128, 64, H], bf, tag="rx")
                with nc.allow_low_precision("softmax recip; bf16 ample"):
                    nc.vector.reciprocal(rx, sx[t])
                x1p = xps.tile([128, H, 64], f32, tag="xp")  # [i, j-half]
                for hl in range(4):
                    c0 = 32 * hl
                    for i in range(H):
                        nc.tensor.matmul(
                            x1p[c0:c0 + 32, i, :],
                            lhsT=VT[:, hl, i, :],
                            rhs=E[hl][:, t * 64:(t + 1) * 64, i],
                            start=(i % 8 == 0),
                            stop=(i % 8 == 7),
                            tile_position=(0, c0),
                        )
                T_sb = tup.tile([128, H, 64], bf, tag="T")
                nc.vector.tensor_tensor(
                    out=T_sb, in0=x1p,
                    in1=rx.rearrange("p j i -> p i j"),
                    op=ALU.mult,
                )
                x2p = xps.tile([128, 64, H], f32, tag="xp")  # [j-half, i]
                for jc2 in range(2):
                    rvc = rqk.tile([128, 4, 32, HD], bf, tag="rq")
                    nc.sync.dma_start(
                        out=rvc,
                        in_=rvt_d[:, :, t * 64 + jc2 * 32:t * 64 + (jc2 + 1) * 32, :],
                        single_packet=True,
                    )
                    for hl in range(4):
                        c0 = 32 * hl
                        for jl in range(32):
                            jh = jc2 * 32 + jl
                            nc.tensor.matmul(
                                x2p[c0:c0 + 32, jh, :],
                                lhsT=rvc[:, hl, jl, :],
                                rhs=E[hl][:, t * 64 + jh, :],
                                start=(jh % 8 == 0),
                                stop=(jh % 8 == 7),
                                tile_position=(0, c0),
                            )
                U_sb = tup.tile([128, 64, H], bf, tag="U")
                nc.vector.tensor_tensor(
                    out=U_sb, in0=x2p, in1=rx, op=ALU.mult,
                )
                nc.vector.tensor_tensor(
                    out=Xn[:, t * 64:(t + 1) * 64, :],
                    in0=U_sb, in1=T_sb.rearrange("p i j -> p j i"),
                    op=ALU.add,
                )

            # ---- phase 5: pair AllGather + output conv -----------------
            xstage = dramp.tile([128, PIX], bf, name="xstage")
            xgather = dramp.tile([2, 128, PIX], bf, name="xgather")
            nc.sync.dma_start(out=xstage, in_=Xn.rearrange("p j i -> p (j i)"))
            nc.gpsimd.collective_compute(
                "AllGather",
                ALU.bypass,
                replica_groups=[[0, 1], [2, 3], [4, 5], [6, 7]],
                ins=[xstage[:]],
                outs=[xgather[:]],
            )
        mid2.close()
        mid.close()

        xgT = xgather.rearrange("g p x -> p g x")
        with tc.tile_pool(name="yop", bufs=1) as yop, \
             tc.tile_pool(name="fps", bufs=4, space="PSUM") as fps:
            yo_sb = [yop.tile([128, PIX], bf, tag=f"yo{ot}", name=f"yo{ot}")
                     for ot in range(2)]
            for px in range(16):
                Xf = xbp.tile([128, 2, 512], bf, tag="xb")
                nc.sync.dma_start(
                    out=Xf, in_=xgT[:, :, px * 512:(px + 1) * 512],
                    single_packet=True,
                )
                for ot in range(2):
                    ps = fps.tile([128, 512], f32, tag="f")
                    for g in range(2):
                        nc.tensor.matmul(
                            ps,
                            lhsT=wo[:, g, ot * 128:(ot + 1) * 128],
                            rhs=Xf[:, g, :],
                            start=(g == 0),
                            stop=(g == 1),
                        )
                    dst = yo_sb[ot][:, px * 512:(px + 1) * 512]
                    if ot == 0:
                        nc.scalar.copy(dst, ps)
                    else:
                        nc.vector.tensor_copy(dst, ps)
            for ot in range(2):
                nc.sync.dma_start(out=yo_d[ot], in_=yo_sb[ot])

    _fix_dma_waits(nc, mybir)
    return nc


def _fix_dma_waits(nc, mybir, max_waits=1):
    """Spill excess sync-waits from DMA instructions onto same-engine NoOps.

    The HW DMA descriptor (DMA_DIRECT2D pseudo) only encodes 2 sync waits;
    Tile's scheduler can emit more (slot-reuse WAR against multiple HWDGE
    completion sems).  Executing the overflow waits on the issuing
    sequencer immediately before the DMA is strictly more conservative,
    so correctness is preserved.
    """
    deny = {"InstAllEngineBarrier", "InstCollectiveCompute", "InstNoOp"}
    fixed = 0
    for fn in nc.m.functions:
        for bb in fn.blocks:
            insts = list(bb.instructions)
            out = []
            for inst in insts:
                tn = type(inst).__name__
                si = getattr(inst, "sync_info", None)
                if (
                    si is not None
                    and tn not in deny
                    and si.on_wait is not None
                    and len(si.on_wait) > max_waits
                ):
                    waits = list(si.on_wait)
                    spill, keep = waits[:-max_waits], waits[-max_waits:]
                    for w in spill:
                        nop = mybir.InstNoOp(
                            name=f"I-waitfix-{fixed}", ins=[], outs=[]
                        )
                        nop.engine = inst.engine
                        nop.sync_info = mybir.SyncInfo(
                            on_wait=[w], on_update=[])
                        nc.register_instruction(nop, overwrite=True)
                        out.append(nop)
                        fixed += 1
                    inst.sync_info = mybir.SyncInfo(
                        on_wait=keep, on_update=list(si.on_update or [])
                    )
                out.append(inst)
            if fixed:
                bb.instructions.clear()
                for x in out:
                    bb.instructions.append(x)
    return fixed


def _host_prep(x, wq, wk, wv, wout, rq, rk, rv, Gq, Gk, Gv1, Gv2):
    import ml_dtypes

    bf16 = ml_dtypes.bfloat16
    gq = np.tanh(Gq.astype(np.float32))
    gk = np.tanh(Gk.astype(np.float32))
    gv1 = np.tanh(Gv1.astype(np.float32))
    gv2 = np.tanh(Gv2.astype(np.float32))

    in_maps = []
    for core in range(NCORES):
        n, hp = core // 2, core % 2
        hs = slice(4 * hp, 4 * hp + 4)
        cs = slice(128 * hp, 128 * hp + 128)

        # x[n] in (chan, j, i) pixel order
        xs = np.ascontiguousarray(
            x[n].transpose(0, 2, 1).reshape(C, PIX)
        ).astype(bf16)
        xs = xs.reshape(2, 128, PIX)

        # stacked conv weights: q rows (/D), k rows, v rows (*gv1)
        row_h = np.repeat(np.arange(4 * hp, 4 * hp + 4), HD)
        wqb = wq[cs] / D
        wkb = wk[cs]
        wvb = wv[cs] * gv1[row_h][:, None]
        wcat = np.concatenate([wqb, wkb, wvb], axis=0).T  # [256, 384]
        wcat = np.ascontiguousarray(wcat.reshape(2, 128, 384)).astype(bf16)

        # relative tables, tanh'd + gate-folded, bf16
        tq = np.tanh(rq[hs].astype(np.float32)) * gq[hs][:, None, None, None]
        tk = np.tanh(rk[hs].astype(np.float32)) * (gk[hs] / D)[:, None, None, None]
        tv = np.tanh(rv[hs].astype(np.float32)) * gv2[hs][:, None, None, None]
        rqs = np.ascontiguousarray(tq.reshape(128, W, W)).astype(bf16)
        rks = np.ascontiguousarray(tk.reshape(128, W, W)).astype(bf16)
        rvt = np.ascontiguousarray(tv.transpose(3, 0, 2, 1)).astype(bf16)  # [w,hl,j,c]

        wot = np.ascontiguousarray(wout.T.reshape(2, 128, 256)).astype(bf16)

        ident = np.tile(np.eye(HD, dtype=np.float32), (4, 1)).astype(bf16)

        in_maps.append({
            "xs": np.asarray(xs),
            "ident": np.asarray(ident),
            "rqs": np.asarray(rqs),
            "rks": np.asarray(rks),
            "rvt": np.asarray(rvt),
            "wcat": np.asarray(wcat),
            "wo": np.asarray(wot),
        })
    return in_maps


def kernel(x, wq, wk, wv, wout, rq, rk, rv, Gq, Gk, Gv1, Gv2, _profile=False):
    from concourse.bass_utils import run_bass_kernel_spmd

    args = [np.asarray(np.asarray(t), np.float32) for t in
            (x, wq, wk, wv, wout, rq, rk, rv, Gq, Gk, Gv1, Gv2)]
    in_maps = _host_prep(*args)

    if "nc" not in _CACHE:
        _CACHE["nc"] = _build_nc()
    nc = _CACHE["nc"]

    res = run_bass_kernel_spmd(
        nc, in_maps, core_ids=list(range(NCORES)), trace=_profile
    )
    out = np.empty((N, C, H, W), np.float32)
    for n in range(N):
        yo = np.asarray(res.results[2 * n]["yo"], np.float32)  # [2, 128, PIX]
        out[n] = yo.reshape(C, W, H).transpose(0, 2, 1)
    if _profile:
        _CACHE["last_result"] = res
    return out
